# revision 17
# baseline (speedup 1.0000x reference)
"""MultiHeadSelectiveAttention TRN2 kernel: FULL inputs -> FULL output.

Shards batch (B=8) across 8 NeuronCores (data-parallel, one batch element
per core). Per batch b, using the value-head-dim-1 collapse:
    v   = x Wv + bv                        [L, H]
    xv  = x^T v                            [D, H]
    ktv = blockdiag_mask(Wk^T xv + bk (x) sum_l v)   [D, H]
    U   = Wq ktv ;  c[h] = bq . ktv[:, h]
    out = sigmoid((x U + c)/8)^T * mask    [H, L]
identical in exact arithmetic to the reference attention.

v5: single-rounded math, fp16 x-path. x streams alone on the sync HWDGE
queue (done ~48us) and is cast f32->fp16 on the vector engine; transposes
run in fp16 (FWL-fast weight loads, 1 cyc/row). Weights stream on the
scalar HWDGE queue (wk prefetched, wq JIT-transposed in phase B). s3/s4
stay f32r; v/xv/z passes are fp16 (half-ulp 2^-11, ~4x tighter than bf16).
Mask applied on host (a no-op for all-ones masks).
"""
import sys
sys.path.insert(0, '/opt/trn_rl_repo')
from contextlib import ExitStack
import numpy as np
import concourse.bass as bass
import concourse.tile as tile
import concourse.mybir as mybir
from concourse.tile import ScopedClock
from concourse.masks import make_identity

f32 = mybir.dt.float32
f32r = mybir.dt.float32r
bf16 = mybir.dt.bfloat16
fp16 = mybir.dt.float16
Copy = mybir.ActivationFunctionType.Copy
Sigmoid = mybir.ActivationFunctionType.Sigmoid

L, D, H = 4096, 1024, 16
NLT, NDT = L // 128, D // 128   # 32, 8
BLK = 4                          # l-tiles per block
NBLK = NLT // BLK                # 8

_wait_fix_counter = [0]
SPLIT_WAITS = [True]

def _split_multi_waits(nc):
    for f in nc.m.functions:
        for bb in f.blocks:
            new_insts = []
            for inst in bb.instructions:
                si = getattr(inst, 'sync_info', None)
                if si is not None and len(si.on_wait) > 1:
                    waits = list(si.on_wait)
                    for w in waits[:-1]:
                        _wait_fix_counter[0] += 1
                        nop = mybir.InstNoOp(
                            name=f"waitfix-{_wait_fix_counter[0]}",
                            engine=inst.engine, opcode="NoOp", ins=[], outs=[],
                            sync_info=mybir.SyncInfo(on_wait=[w], on_update=[]),
                        )
                        new_insts.append(nop)
                    inst.sync_info = mybir.SyncInfo(
                        on_wait=[waits[-1]], on_update=list(si.on_update))
                new_insts.append(inst)
            bb.instructions[:] = new_insts

def _drain_and_barrier_split(self, tick_clock, wait_clock):
    nc = self.nc
    probe = nc.sync.nop()
    wait_clock.add_sem_waits(probe.ins, ScopedClock({None: tick_clock.global_clock}))
    nc.sync.drain()
    nc.all_engine_barrier()
    assert self.sems is not None
    popped = nc._tile_sem_poison_stack.pop()
    assert popped is self._sem_poison
    nc.clear_and_free_semaphores(list(self.sems.allocated().values()))
    nc.all_engine_barrier()
    if SPLIT_WAITS[0]:
        _split_multi_waits(nc)

tile.TileContext._drain_and_barrier = _drain_and_barrier_split


def build():
    nc = bass.Bass(trn_type="TRN2")
    x = nc.dram_tensor("x", [L, D], f32r, kind="ExternalInput")
    wk = nc.dram_tensor("wk", [D, D], f32r, kind="ExternalInput")
    wq = nc.dram_tensor("wq", [D, D], f32r, kind="ExternalInput")
    # cst packs: cols 0:128 Wv d-tiles; 128:144 bq (dup pairs);
    # 144:160 bv bcast; col 160 rows 0:16 bv column
    cst = nc.dram_tensor("cst", [128, 176], f32, kind="ExternalInput")
    bkb = nc.dram_tensor("bkb", [H, D], f32, kind="ExternalInput")
    bdm = nc.dram_tensor("bdm", [H, D], f32, kind="ExternalInput")
    out = nc.dram_tensor("out", [H, L], f32, kind="ExternalOutput")

    with ExitStack() as ctx:
        tc = ctx.enter_context(tile.TileContext(nc))
        konst = ctx.enter_context(tc.tile_pool(name="konst", bufs=1))
        xtrp = ctx.enter_context(tc.tile_pool(name="xtr", bufs=1))
        pers = ctx.enter_context(tc.tile_pool(name="pers", bufs=1))
        wkp = ctx.enter_context(tc.tile_pool(name="wkp", bufs=1))
        ps_xv = ctx.enter_context(tc.tile_pool(name="ps_xv", bufs=1, space="PSUM"))

        # ---------------- constants ----------------
        cstt = konst.tile([128, 176], f32)
        nc.scalar.dma_start(cstt[:], cst[:, :])
        ident = konst.tile([128, 128], f32)
        make_identity(nc, ident[:])
        identh = konst.tile([128, 128], fp16)
        nc.vector.tensor_copy(identh[:], ident[:])
        # wk/wqh tiles filled during phase A (DMA slack)
        wkt = [wkp.tile([128, D], f32r, name=f"wkp{k}", tag=f"wkt{k}")
               for k in range(NDT)]
        wqtp = ctx.enter_context(tc.tile_pool(name="wqtp", bufs=1))
        wqt = [wqtp.tile([128, D], fp16, name=f"wqt{c}", tag=f"wqt{c}")
               for c in range(NDT)]
        wvh = konst.tile([128, 128], fp16)
        nc.vector.tensor_copy(wvh[:], cstt[:, 0:128])
        wv_d = [wvh[:, 16 * k:16 * k + 16] for k in range(NDT)]
        bqc = konst.tile([128, 16], fp16)
        nc.vector.tensor_copy(bqc[:], cstt[:, 128:144])
        bvtb = konst.tile([128, H], fp16)
        nc.vector.tensor_copy(bvtb[:], cstt[:, 144:160])

        # persistent: fp16 x^T for the v and z passes
        xtr = [xtrp.tile([128, L], fp16, name=f"xtr{d}", tag=f"xtr{d}")
               for d in range(NDT)]
        xv_ps = [ps_xv.tile([16, 512], f32, name=f"xv{c}", tag=f"xv{c}")
                 for c in range(2)]
        prep = ctx.enter_context(tc.tile_pool(name="prep", bufs=1))
        bkt = pers.tile([H, D], f32)
        nc.scalar.dma_start(bkt[:], bkb[:, :])
        bdmT = prep.tile([H, D], f32, name="bdmT", tag="bdmT")
        nc.scalar.dma_start(bdmT[:], bdm[:, :])
        nxv = [0, 0]
        svps = []

        # ---------------- PHASE A ----------------
        with tc.tile_pool(name="xnatp", bufs=3) as xnatp, \
             tc.tile_pool(name="xhp", bufs=2) as xhp, \
             tc.tile_pool(name="wqnp", bufs=2) as wqnp, \
             tc.tile_pool(name="sbA", bufs=2) as sbA, \
             tc.tile_pool(name="vnp", bufs=3) as vnp, \
             tc.tile_pool(name="ps_tr", bufs=2, space="PSUM") as ps_tr, \
             tc.tile_pool(name="ps_q", bufs=1, space="PSUM") as ps_q, \
             tc.tile_pool(name="ps_v", bufs=2, space="PSUM") as ps_v, \
             tc.tile_pool(name="ps_f", bufs=1, space="PSUM") as ps_f:
            prev_p2 = [None]

            def p2_step(m):
                # one of 8 P2 matmuls for the previous block
                pblk, vsum_p, xh_p = prev_p2[0]
                j, c = m // 2, m % 2
                nxv[c] += 1
                nc.tensor.matmul(
                    xv_ps[c][:],
                    vsum_p[:, 16 * j:16 * j + 16],
                    xh_p[:, D * j + 512 * c:D * j + 512 * c + 512],
                    start=(nxv[c] == 1), stop=(nxv[c] == NLT))

            for blk in range(NBLK):
                xblk = xnatp.tile([128, BLK * D], f32r, tag="xnat")
                nc.sync.dma_start(
                    xblk[:].rearrange("p (j d) -> p j d", j=BLK),
                    x[512 * blk:512 * blk + 512, :]
                    .rearrange("(j p) d -> p j d", p=128))
                # weight prefetch in the block's DMA slack (scalar queue)
                nc.scalar.dma_start(wkt[blk][:], wk[128 * blk:128 * blk + 128, :])
                wc = wqnp.tile([128, D], f32r, tag="wqc")
                nc.scalar.dma_start(wc[:], wq[128 * blk:128 * blk + 128, :])
                wch = wqnp.tile([128, D], fp16, tag="wqch")
                nc.vector.tensor_copy(wch[:], wc[:].bitcast(f32))
                xh = xhp.tile([128, BLK * D], fp16, tag="xh")
                for j in range(BLK):
                    nc.vector.tensor_copy(
                        xh[:, D * j:D * (j + 1)],
                        xblk[:, D * j:D * (j + 1)].bitcast(f32))
                xnat = [xh[:, D * j:D * (j + 1)] for j in range(BLK)]
                # x transposes (fp16) with P1 of this block lagged 2 d-groups
                # and P2 of the previous block interleaved (keeps PE warm)
                psv = ps_v.tile([16, 512], f32, tag="v")
                def p1_step(d):
                    nc.tensor.matmul(
                        psv[:], wv_d[d], xtr[d][:, 512 * blk:512 * blk + 512],
                        start=(d == 0), stop=(d == NDT - 1))
                for d in range(NDT):
                    ps = ps_tr.tile([128, 512], fp16, tag="tr")
                    for j in range(BLK):
                        nc.tensor.matmul(
                            ps[:, 128 * j:128 * j + 128],
                            xnat[j][:, 128 * d:128 * d + 128],
                            identh[:],
                            start=True, stop=True, is_transpose=True,
                            skip_group_check=True)
                    if d % 2 == 0:
                        nc.scalar.copy(xtr[d][:, 512 * blk:512 * blk + 512], ps[:])
                    else:
                        nc.vector.tensor_copy(
                            xtr[d][:, 512 * blk:512 * blk + 512], ps[:])
                    psw = ps_q.tile([128, 128], fp16, tag="qw")
                    nc.tensor.matmul(
                        psw[:], wch[:, 128 * d:128 * d + 128], identh[:],
                        start=True, stop=True, is_transpose=True,
                        skip_group_check=True)
                    if d % 2 == 0:
                        nc.vector.tensor_copy(
                            wqt[d][:, 128 * blk:128 * blk + 128], psw[:])
                    else:
                        nc.scalar.copy(
                            wqt[d][:, 128 * blk:128 * blk + 128], psw[:])
                    if prev_p2[0] is not None:
                        p2_step(d)
                    if d >= 3:
                        p1_step(d - 3)
                p1_step(NDT - 3)
                p1_step(NDT - 2)
                p1_step(NDT - 1)
                vts = sbA.tile([16, 512], fp16, tag="vts")
                svp = sbA.tile([16, 1], f32, name="svp", tag=f"svp{blk}", bufs=1)
                nc.scalar.activation(vts[:], psv[:], Copy, accum_out=svp[:])
                svps.append(svp)
                # fold-transpose to v natural [128, 16] per l-tile
                psf = ps_f.tile([128, BLK * 16], fp16, tag="vf")
                for j in range(BLK):
                    nc.tensor.matmul(
                        psf[:, 16 * j:16 * j + 16],
                        vts[:, 128 * j:128 * j + 128],
                        identh[0:16, 0:16],
                        start=True, stop=True, is_transpose=True,
                        skip_group_check=True)
                vsum = vnp.tile([128, BLK * 16], fp16, tag="vsum")
                vs3 = vsum[:].rearrange("p (j h) -> p j h", j=BLK)
                pf3 = psf[:].rearrange("p (j h) -> p j h", j=BLK)
                nc.vector.tensor_add(
                    vs3, pf3, bvtb[:].unsqueeze(1).broadcast_to([128, BLK, H]))
                prev_p2[0] = (blk, vsum, xh)
            for m in range(8):
                p2_step(m)

        # xv drain + sv
        xvt = pers.tile([H, D], f32, tag="xvt")
        for c in range(2):
            nc.scalar.copy(xvt[:, 512 * c:512 * c + 512], xv_ps[c][:])
        sv = pers.tile([H, 1], f32, tag="sv")
        nc.vector.tensor_add(sv[:], svps[0][:], svps[1][:])
        for b in range(2, NBLK):
            nc.vector.tensor_add(sv[:], sv[:], svps[b][:])
        bvL = pers.tile([H, 1], f32, tag="bvL")
        nc.scalar.mul(bvL[:], cstt[0:16, 160:161], float(L))
        nc.vector.tensor_add(sv[:], sv[:], bvL[:])

        # ---------------- PHASE B ----------------
        with tc.tile_pool(name="sbB", bufs=2) as sbB:
            # xv natural tiles [128, 16] f32r per d-tile
            xvp = []
            with tc.tile_pool(name="ps_m1", bufs=2, space="PSUM") as ps_m1:
                for k in range(NDT):
                    psm = ps_m1.tile([128, 16], f32, tag="m1")
                    nc.tensor.matmul(
                        psm[:], xvt[:, 128 * k:128 * k + 128], ident[0:16, 0:16],
                        start=True, stop=True, is_transpose=True,
                        skip_group_check=True)
                    p = prep.tile([128, 16], f32r, name=f"xvp{k}", tag=f"xvp{k}")
                    nc.scalar.copy(p[:], psm[:])
                    xvp.append(p)
            # s3: ktv_full^T = xv^T Wk (wk prefetched)
            with tc.tile_pool(name="ps_s3", bufs=1, space="PSUM") as ps_s3:
                ps3 = [ps_s3.tile([16, 512], f32, name=f"s3{c}", tag=f"s3{c}")
                       for c in range(2)]
                for k in range(NDT):
                    for c in range(2):
                        nc.tensor.matmul(
                            ps3[c][:], xvp[k][:],
                            wkt[k][:, 512 * c:512 * c + 512],
                            start=(k == 0), stop=(k == NDT - 1))
                ktvt = sbB.tile([H, D], f32, tag="ktvt", bufs=1)
                nc.scalar.activation(ktvt[:], bkt[:], Copy, scale=sv[:])
                for c in range(2):
                    sl = ktvt[:, 512 * c:512 * c + 512]
                    nc.vector.tensor_add(sl, sl, ps3[c][:])
                nc.vector.tensor_mul(ktvt[:], ktvt[:], bdmT[:])
            # ktv_bd natural tiles + c accumulation
            ktvp = []
            with tc.tile_pool(name="ps_m2", bufs=2, space="PSUM") as ps_m2, \
                 tc.tile_pool(name="ps_c", bufs=1, space="PSUM") as ps_c:
                for k in range(NDT):
                    psm = ps_m2.tile([128, 16], f32, tag="m2")
                    nc.tensor.matmul(
                        psm[:], ktvt[:, 128 * k:128 * k + 128], ident[0:16, 0:16],
                        start=True, stop=True, is_transpose=True,
                        skip_group_check=True)
                    p = prep.tile([128, 16], fp16, name=f"ktvp{k}", tag=f"ktvp{k}")
                    nc.scalar.copy(p[:], psm[:])
                    ktvp.append(p)
                psc = ps_c.tile([16, 2], f32, tag="c", bufs=1)
                for k in range(NDT):
                    nc.tensor.matmul(
                        psc[:], ktvp[k][:], bqc[:, 2 * k:2 * k + 2],
                        start=(k == 0), stop=(k == NDT - 1))
                cdiv8 = pers.tile([H, 1], f32, tag="cdiv8")
                nc.scalar.mul(cdiv8[:], psc[:, 0:1], 0.125)
            # s4: U^T accumulation (wqt resident from phase A)
            with tc.tile_pool(name="ps_s4", bufs=1, space="PSUM") as ps_s4:
                ps4 = [ps_s4.tile([16, 512], f32, name=f"s4{c}", tag=f"s4{c}")
                       for c in range(2)]
                for b in range(NDT):
                    for c in range(2):
                        nc.tensor.matmul(
                            ps4[c][:], ktvp[b][:],
                            wqt[b][:, 512 * c:512 * c + 512],
                            start=(b == 0), stop=(b == NDT - 1))
                ut = sbB.tile([H, D], f32, name="ut", tag="ut", bufs=1)
                for c in range(2):
                    nc.scalar.copy(ut[:, 512 * c:512 * c + 512], ps4[c][:])
            # U natural tiles [128, 16] per d-tile (bf16 for the z pass)
            upr = []
            with tc.tile_pool(name="ps_m3", bufs=2, space="PSUM") as ps_m3:
                for d in range(NDT):
                    psm = ps_m3.tile([128, 16], f32, tag="m3")
                    nc.tensor.matmul(
                        psm[:], ut[:, 128 * d:128 * d + 128], ident[0:16, 0:16],
                        start=True, stop=True, is_transpose=True,
                        skip_group_check=True)
                    p = prep.tile([128, 16], fp16, name=f"upr{d}", tag=f"upr{d}")
                    nc.vector.tensor_copy(p[:], psm[:])
                    upr.append(p)
            # P5: z^T chunks + sigmoid + store
            with tc.tile_pool(name="ps_5", bufs=2, space="PSUM") as ps_5:
                for ch in range(8):
                    ps5 = ps_5.tile([16, 512], f32, tag="s5")
                    for d in range(NDT):
                        nc.tensor.matmul(
                            ps5[:], upr[d][:], xtr[d][:, 512 * ch:512 * ch + 512],
                            start=(d == 0), stop=(d == NDT - 1))
                    sg = sbB.tile([H, 512], f32, name="sg", tag="sg")
                    nc.scalar.activation(sg[:], ps5[:], Sigmoid,
                                         bias=cdiv8[:], scale=0.125)
                    nc.sync.dma_start(out[:, 512 * ch:512 * ch + 512], sg[:])
    return nc


def ref_numpy(x, wq, wk, wv, bq, bk, bv):
    """f64 reference of the decomposed math for per-stage validation."""
    x64 = x.astype(np.float64)
    v = x64 @ wv.astype(np.float64) + bv.astype(np.float64)   # [L, H]
    xv = x64.T @ v                                            # [D, H]
    ktvfull = wk.astype(np.float64).T @ xv                    # [D, H]
    sv = v.sum(axis=0)                                        # [H]
    ktvfull = ktvfull + np.outer(bk.astype(np.float64), sv)
    bd = np.zeros((D, H))
    for h in range(H):
        bd[64 * h:64 * h + 64, h] = 1.0
    ktvbd = ktvfull * bd
    u = wq.astype(np.float64) @ ktvbd                         # [D, H]
    c = bq.astype(np.float64) @ ktvbd                         # [H]
    z = (x64 @ u + c) / 8.0                                   # [L, H]
    p = 1.0 / (1.0 + np.exp(-z))
    return dict(v=v, xvt=xv.T, ktvbdt=ktvbd.T, ut=u.T, c=c / 8.0, out=p.T)


B = 8
_BDM = np.zeros((H, D), dtype=np.float32)
for _h in range(H):
    _BDM[_h, 64 * _h:64 * _h + 64] = 1.0
_cache = {}

def _get_nc(xbf16=True):
    if "nc" not in _cache:
        _cache["nc"] = build()
    return _cache["nc"]


def _make_cst(Wq, Wk, Wv, bq, bk, bv):
    cstm = np.zeros((128, 176), dtype=np.float32)
    for k in range(NDT):
        cstm[:, 16 * k:16 * k + 16] = Wv[128 * k:128 * k + 128, :]
    bqt = bq.reshape(8, 128).T
    cstm[:, 128:144:2] = bqt
    cstm[:, 129:144:2] = bqt
    cstm[:, 144:160] = np.broadcast_to(bv[None, :], (128, 16))
    cstm[0:16, 160] = bv
    return np.ascontiguousarray(cstm)


def kernel(x, mask, Wq, bq, Wk, bk, Wv, bv, xbf16=True):
    from concourse.bass_utils import run_bass_kernel_spmd
    x = np.asarray(x, dtype=np.float32)
    mask_f = np.asarray(mask).astype(np.float32)
    Wq = np.ascontiguousarray(np.asarray(Wq, dtype=np.float32))
    Wk = np.ascontiguousarray(np.asarray(Wk, dtype=np.float32))
    Wv = np.ascontiguousarray(np.asarray(Wv, dtype=np.float32))
    bq = np.asarray(bq, dtype=np.float32)
    bk = np.asarray(bk, dtype=np.float32)
    bv = np.asarray(bv, dtype=np.float32)
    nc = _get_nc()
    cstm = _make_cst(Wq, Wk, Wv, bq, bk, bv)
    bkb_ = np.ascontiguousarray(np.broadcast_to(bk[None, :], (H, D)))
    in_maps = []
    for b in range(B):
        in_maps.append({
            "x": np.ascontiguousarray(x[b]),
            "wq": Wq, "wk": Wk,
            "cst": cstm, "bkb": bkb_, "bdm": _BDM,
        })
    res = run_bass_kernel_spmd(nc, in_maps, core_ids=list(range(B)))
    out = np.stack([res.results[b]["out"] for b in range(B)], axis=0)
    out = out * mask_f[:, None, :]
    return out.astype(np.float32)


# revision 18
# speedup vs baseline: 1.1643x; 1.1643x over previous
"""MultiHeadSelectiveAttention TRN2 kernel: FULL inputs -> FULL output.

Shards batch (B=8) across 8 NeuronCores (data-parallel, one batch element
per core). Per batch b, using the value-head-dim-1 collapse:
    v   = x Wv + bv                        [L, H]
    xv  = x^T v                            [D, H]
    ktv = blockdiag_mask(Wk^T xv + bk (x) sum_l v)   [D, H]
    U   = Wq ktv ;  c[h] = bq . ktv[:, h]
    out = sigmoid((x U + c)/8)^T * mask    [H, L]
identical in exact arithmetic to the reference attention.

v5: single-rounded math, fp16 x-path. x streams alone on the sync HWDGE
queue (done ~48us) and is cast f32->fp16 on the vector engine; transposes
run in fp16 (FWL-fast weight loads, 1 cyc/row). Weights stream on the
scalar HWDGE queue (wk prefetched, wq JIT-transposed in phase B). s3/s4
stay f32r; v/xv/z passes are fp16 (half-ulp 2^-11, ~4x tighter than bf16).
Mask applied on host (a no-op for all-ones masks).
"""
import sys
sys.path.insert(0, '/opt/trn_rl_repo')
from contextlib import ExitStack
import numpy as np
import concourse.bass as bass
import concourse.tile as tile
import concourse.mybir as mybir
from concourse.tile import ScopedClock
from concourse.masks import make_identity

f32 = mybir.dt.float32
f32r = mybir.dt.float32r
bf16 = mybir.dt.bfloat16
fp16 = mybir.dt.float16
Copy = mybir.ActivationFunctionType.Copy
Sigmoid = mybir.ActivationFunctionType.Sigmoid

L, D, H = 4096, 1024, 16
NLT, NDT = L // 128, D // 128   # 32, 8
BLK = 4                          # l-tiles per block
NBLK = NLT // BLK                # 8

_wait_fix_counter = [0]
SPLIT_WAITS = [True]

def _split_multi_waits(nc):
    for f in nc.m.functions:
        for bb in f.blocks:
            new_insts = []
            for inst in bb.instructions:
                si = getattr(inst, 'sync_info', None)
                if si is not None and len(si.on_wait) > 1:
                    waits = list(si.on_wait)
                    for w in waits[:-1]:
                        _wait_fix_counter[0] += 1
                        nop = mybir.InstNoOp(
                            name=f"waitfix-{_wait_fix_counter[0]}",
                            engine=inst.engine, opcode="NoOp", ins=[], outs=[],
                            sync_info=mybir.SyncInfo(on_wait=[w], on_update=[]),
                        )
                        new_insts.append(nop)
                    inst.sync_info = mybir.SyncInfo(
                        on_wait=[waits[-1]], on_update=list(si.on_update))
                new_insts.append(inst)
            bb.instructions[:] = new_insts

def _drain_and_barrier_split(self, tick_clock, wait_clock):
    nc = self.nc
    probe = nc.sync.nop()
    wait_clock.add_sem_waits(probe.ins, ScopedClock({None: tick_clock.global_clock}))
    nc.sync.drain()
    nc.all_engine_barrier()
    assert self.sems is not None
    popped = nc._tile_sem_poison_stack.pop()
    assert popped is self._sem_poison
    nc.clear_and_free_semaphores(list(self.sems.allocated().values()))
    nc.all_engine_barrier()
    if SPLIT_WAITS[0]:
        _split_multi_waits(nc)

tile.TileContext._drain_and_barrier = _drain_and_barrier_split


def build():
    nc = bass.Bass(trn_type="TRN2")
    x = nc.dram_tensor("x", [L, D], f32r, kind="ExternalInput")
    wk = nc.dram_tensor("wk", [D, D], f32r, kind="ExternalInput")
    wq = nc.dram_tensor("wq", [D, D], f32r, kind="ExternalInput")
    # cst packs: cols 0:128 Wv d-tiles; 128:144 bq (dup pairs);
    # 144:160 bv bcast; col 160 rows 0:16 bv column
    cst = nc.dram_tensor("cst", [128, 176], f32, kind="ExternalInput")
    bkb = nc.dram_tensor("bkb", [H, D], f32, kind="ExternalInput")
    bdm = nc.dram_tensor("bdm", [H, D], f32, kind="ExternalInput")
    out = nc.dram_tensor("out", [H, L], f32, kind="ExternalOutput")

    with ExitStack() as ctx:
        tc = ctx.enter_context(tile.TileContext(nc))
        konst = ctx.enter_context(tc.tile_pool(name="konst", bufs=1))
        xtrp = ctx.enter_context(tc.tile_pool(name="xtr", bufs=1))
        pers = ctx.enter_context(tc.tile_pool(name="pers", bufs=1))
        wkp = ctx.enter_context(tc.tile_pool(name="wkp", bufs=1))
        ps_xv = ctx.enter_context(tc.tile_pool(name="ps_xv", bufs=1, space="PSUM"))

        # ---------------- constants ----------------
        cstt = konst.tile([128, 176], f32)
        nc.scalar.dma_start(cstt[:], cst[:, :])
        ident = konst.tile([128, 128], f32)
        make_identity(nc, ident[:])
        identh = konst.tile([128, 128], fp16)
        nc.vector.tensor_copy(identh[:], ident[:])
        # wk/wqh tiles filled during phase A (DMA slack)
        wkt = [wkp.tile([128, D], f32r, name=f"wkp{k}", tag=f"wkt{k}")
               for k in range(NDT)]
        wqtp = ctx.enter_context(tc.tile_pool(name="wqtp", bufs=1))
        wqt = [wqtp.tile([128, D], fp16, name=f"wqt{c}", tag=f"wqt{c}")
               for c in range(NDT)]
        wvh = konst.tile([128, 128], fp16)
        nc.vector.tensor_copy(wvh[:], cstt[:, 0:128])
        wv_d = [wvh[:, 16 * k:16 * k + 16] for k in range(NDT)]
        bqc = konst.tile([128, 16], fp16)
        nc.vector.tensor_copy(bqc[:], cstt[:, 128:144])
        bvtb = konst.tile([128, H], fp16)
        nc.vector.tensor_copy(bvtb[:], cstt[:, 144:160])

        # persistent: fp16 x^T for the v and z passes
        xtr = [xtrp.tile([128, L], fp16, name=f"xtr{d}", tag=f"xtr{d}")
               for d in range(NDT)]
        xv_ps = [ps_xv.tile([16, 512], f32, name=f"xv{c}", tag=f"xv{c}")
                 for c in range(2)]
        prep = ctx.enter_context(tc.tile_pool(name="prep", bufs=1))
        bkt = pers.tile([H, D], f32)
        nc.scalar.dma_start(bkt[:], bkb[:, :])
        bdmT = prep.tile([H, D], f32, name="bdmT", tag="bdmT")
        nc.scalar.dma_start(bdmT[:], bdm[:, :])
        nxv = [0, 0]
        svps = []

        # ---------------- PHASE A ----------------
        with tc.tile_pool(name="xnatp", bufs=3) as xnatp, \
             tc.tile_pool(name="xhp", bufs=2) as xhp, \
             tc.tile_pool(name="wqnp", bufs=2) as wqnp, \
             tc.tile_pool(name="sbA", bufs=2) as sbA, \
             tc.tile_pool(name="vnp", bufs=3) as vnp, \
             tc.tile_pool(name="ps_tr", bufs=2, space="PSUM") as ps_tr, \
             tc.tile_pool(name="ps_q", bufs=2, space="PSUM") as ps_q, \
             tc.tile_pool(name="ps_v", bufs=1, space="PSUM") as ps_v, \
             tc.tile_pool(name="ps_f", bufs=1, space="PSUM") as ps_f:
            prev_p2 = [None]

            def p2_step(m):
                # one of 8 P2 matmuls for the previous block
                pblk, vsum_p, xh_p = prev_p2[0]
                j, c = m // 2, m % 2
                nxv[c] += 1
                nc.tensor.matmul(
                    xv_ps[c][:],
                    vsum_p[:, 16 * j:16 * j + 16],
                    xh_p[:, D * j + 512 * c:D * j + 512 * c + 512],
                    start=(nxv[c] == 1), stop=(nxv[c] == NLT))

            for blk in range(NBLK):
                xblk = xnatp.tile([128, BLK * D], f32r, tag="xnat")
                nc.sync.dma_start(
                    xblk[:].rearrange("p (j d) -> p j d", j=BLK),
                    x[512 * blk:512 * blk + 512, :]
                    .rearrange("(j p) d -> p j d", p=128))
                # weight prefetch in the block's DMA slack (scalar queue)
                nc.scalar.dma_start(wkt[blk][:], wk[128 * blk:128 * blk + 128, :])
                wc = wqnp.tile([128, D], f32r, tag="wqc")
                nc.scalar.dma_start(wc[:], wq[128 * blk:128 * blk + 128, :])
                wch = wqnp.tile([128, D], fp16, tag="wqch")
                nc.vector.tensor_copy(wch[:], wc[:].bitcast(f32))
                xh = xhp.tile([128, BLK * D], fp16, tag="xh")
                for j in range(BLK):
                    nc.vector.tensor_copy(
                        xh[:, D * j:D * (j + 1)],
                        xblk[:, D * j:D * (j + 1)].bitcast(f32))
                xnat = [xh[:, D * j:D * (j + 1)] for j in range(BLK)]
                # x transposes (fp16) with P1 of this block lagged 2 d-groups
                # and P2 of the previous block interleaved (keeps PE warm)
                psv = ps_v.tile([16, 512], f32, tag="v")
                def p1_step(d):
                    nc.tensor.matmul(
                        psv[:], wv_d[d], xtr[d][:, 512 * blk:512 * blk + 512],
                        start=(d == 0), stop=(d == NDT - 1))
                for d in range(NDT):
                    ps = ps_tr.tile([128, 512], fp16, tag="tr")
                    for j in range(BLK):
                        nc.tensor.matmul(
                            ps[:, 128 * j:128 * j + 128],
                            xnat[j][:, 128 * d:128 * d + 128],
                            identh[:],
                            start=True, stop=True, is_transpose=True,
                            skip_group_check=True)
                    if d % 2 == 0:
                        nc.scalar.copy(xtr[d][:, 512 * blk:512 * blk + 512], ps[:])
                    else:
                        nc.vector.tensor_copy(
                            xtr[d][:, 512 * blk:512 * blk + 512], ps[:])
                    psw = ps_q.tile([128, 128], fp16, tag="qw")
                    nc.tensor.matmul(
                        psw[:], wch[:, 128 * d:128 * d + 128], identh[:],
                        start=True, stop=True, is_transpose=True,
                        skip_group_check=True)
                    if d % 2 == 0:
                        nc.vector.tensor_copy(
                            wqt[d][:, 128 * blk:128 * blk + 128], psw[:])
                    else:
                        nc.scalar.copy(
                            wqt[d][:, 128 * blk:128 * blk + 128], psw[:])
                    if prev_p2[0] is not None:
                        p2_step(d)
                    if d >= 3:
                        p1_step(d - 3)
                p1_step(NDT - 3)
                p1_step(NDT - 2)
                p1_step(NDT - 1)
                vts = sbA.tile([16, 512], fp16, tag="vts")
                svp = sbA.tile([16, 1], f32, name="svp", tag=f"svp{blk}", bufs=1)
                nc.scalar.activation(vts[:], psv[:], Copy, accum_out=svp[:])
                svps.append(svp)
                # fold-transpose to v natural [128, 16] per l-tile
                psf = ps_f.tile([128, BLK * 16], fp16, tag="vf")
                for j in range(BLK):
                    nc.tensor.matmul(
                        psf[:, 16 * j:16 * j + 16],
                        vts[:, 128 * j:128 * j + 128],
                        identh[0:16, 0:16],
                        start=True, stop=True, is_transpose=True,
                        skip_group_check=True)
                vsum = vnp.tile([128, BLK * 16], fp16, tag="vsum")
                vs3 = vsum[:].rearrange("p (j h) -> p j h", j=BLK)
                pf3 = psf[:].rearrange("p (j h) -> p j h", j=BLK)
                nc.vector.tensor_add(
                    vs3, pf3, bvtb[:].unsqueeze(1).broadcast_to([128, BLK, H]))
                prev_p2[0] = (blk, vsum, xh)
            for m in range(8):
                p2_step(m)

        # xv drain + sv
        xvt = pers.tile([H, D], f32, tag="xvt")
        for c in range(2):
            nc.scalar.copy(xvt[:, 512 * c:512 * c + 512], xv_ps[c][:])
        sv = pers.tile([H, 1], f32, tag="sv")
        nc.vector.tensor_add(sv[:], svps[0][:], svps[1][:])
        for b in range(2, NBLK):
            nc.vector.tensor_add(sv[:], sv[:], svps[b][:])
        bvL = pers.tile([H, 1], f32, tag="bvL")
        nc.scalar.mul(bvL[:], cstt[0:16, 160:161], float(L))
        nc.vector.tensor_add(sv[:], sv[:], bvL[:])

        # ---------------- PHASE B ----------------
        with tc.tile_pool(name="sbB", bufs=2) as sbB:
            # xv natural tiles [128, 16] f32r per d-tile
            xvp = []
            with tc.tile_pool(name="ps_m1", bufs=2, space="PSUM") as ps_m1:
                for k in range(NDT):
                    psm = ps_m1.tile([128, 16], f32, tag="m1")
                    nc.tensor.matmul(
                        psm[:], xvt[:, 128 * k:128 * k + 128], ident[0:16, 0:16],
                        start=True, stop=True, is_transpose=True,
                        skip_group_check=True)
                    p = prep.tile([128, 16], f32r, name=f"xvp{k}", tag=f"xvp{k}")
                    nc.scalar.copy(p[:], psm[:])
                    xvp.append(p)
            # s3: ktv_full^T = xv^T Wk (wk prefetched)
            with tc.tile_pool(name="ps_s3", bufs=1, space="PSUM") as ps_s3:
                ps3 = [ps_s3.tile([16, 512], f32, name=f"s3{c}", tag=f"s3{c}")
                       for c in range(2)]
                for k in range(NDT):
                    for c in range(2):
                        nc.tensor.matmul(
                            ps3[c][:], xvp[k][:],
                            wkt[k][:, 512 * c:512 * c + 512],
                            start=(k == 0), stop=(k == NDT - 1))
                ktvt = sbB.tile([H, D], f32, tag="ktvt", bufs=1)
                nc.scalar.activation(ktvt[:], bkt[:], Copy, scale=sv[:])
                for c in range(2):
                    sl = ktvt[:, 512 * c:512 * c + 512]
                    nc.vector.tensor_add(sl, sl, ps3[c][:])
                nc.vector.tensor_mul(ktvt[:], ktvt[:], bdmT[:])
            # ktv_bd natural tiles + c accumulation
            ktvp = []
            with tc.tile_pool(name="ps_m2", bufs=2, space="PSUM") as ps_m2, \
                 tc.tile_pool(name="ps_c", bufs=1, space="PSUM") as ps_c:
                for k in range(NDT):
                    psm = ps_m2.tile([128, 16], f32, tag="m2")
                    nc.tensor.matmul(
                        psm[:], ktvt[:, 128 * k:128 * k + 128], ident[0:16, 0:16],
                        start=True, stop=True, is_transpose=True,
                        skip_group_check=True)
                    p = prep.tile([128, 16], fp16, name=f"ktvp{k}", tag=f"ktvp{k}")
                    nc.scalar.copy(p[:], psm[:])
                    ktvp.append(p)
                psc = ps_c.tile([16, 2], f32, tag="c", bufs=1)
                for k in range(NDT):
                    nc.tensor.matmul(
                        psc[:], ktvp[k][:], bqc[:, 2 * k:2 * k + 2],
                        start=(k == 0), stop=(k == NDT - 1))
                cdiv8 = pers.tile([H, 1], f32, tag="cdiv8")
                nc.scalar.mul(cdiv8[:], psc[:, 0:1], 0.125)
            # s4: U^T accumulation (wqt resident from phase A)
            with tc.tile_pool(name="ps_s4", bufs=1, space="PSUM") as ps_s4:
                ps4 = [ps_s4.tile([16, 512], f32, name=f"s4{c}", tag=f"s4{c}")
                       for c in range(2)]
                for b in range(NDT):
                    for c in range(2):
                        nc.tensor.matmul(
                            ps4[c][:], ktvp[b][:],
                            wqt[b][:, 512 * c:512 * c + 512],
                            start=(b == 0), stop=(b == NDT - 1))
                ut = sbB.tile([H, D], f32, name="ut", tag="ut", bufs=1)
                for c in range(2):
                    nc.scalar.copy(ut[:, 512 * c:512 * c + 512], ps4[c][:])
            # U natural tiles [128, 16] per d-tile (bf16 for the z pass)
            upr = []
            with tc.tile_pool(name="ps_m3", bufs=2, space="PSUM") as ps_m3:
                for d in range(NDT):
                    psm = ps_m3.tile([128, 16], f32, tag="m3")
                    nc.tensor.matmul(
                        psm[:], ut[:, 128 * d:128 * d + 128], ident[0:16, 0:16],
                        start=True, stop=True, is_transpose=True,
                        skip_group_check=True)
                    p = prep.tile([128, 16], fp16, name=f"upr{d}", tag=f"upr{d}")
                    nc.vector.tensor_copy(p[:], psm[:])
                    upr.append(p)
            # P5: z^T chunks + sigmoid + store
            with tc.tile_pool(name="ps_5", bufs=2, space="PSUM") as ps_5:
                for ch in range(8):
                    ps5 = ps_5.tile([16, 512], f32, tag="s5")
                    for d in range(NDT):
                        nc.tensor.matmul(
                            ps5[:], upr[d][:], xtr[d][:, 512 * ch:512 * ch + 512],
                            start=(d == 0), stop=(d == NDT - 1))
                    sg = sbB.tile([H, 512], f32, name="sg", tag="sg")
                    nc.scalar.activation(sg[:], ps5[:], Sigmoid,
                                         bias=cdiv8[:], scale=0.125)
                    nc.sync.dma_start(out[:, 512 * ch:512 * ch + 512], sg[:])
    return nc


def ref_numpy(x, wq, wk, wv, bq, bk, bv):
    """f64 reference of the decomposed math for per-stage validation."""
    x64 = x.astype(np.float64)
    v = x64 @ wv.astype(np.float64) + bv.astype(np.float64)   # [L, H]
    xv = x64.T @ v                                            # [D, H]
    ktvfull = wk.astype(np.float64).T @ xv                    # [D, H]
    sv = v.sum(axis=0)                                        # [H]
    ktvfull = ktvfull + np.outer(bk.astype(np.float64), sv)
    bd = np.zeros((D, H))
    for h in range(H):
        bd[64 * h:64 * h + 64, h] = 1.0
    ktvbd = ktvfull * bd
    u = wq.astype(np.float64) @ ktvbd                         # [D, H]
    c = bq.astype(np.float64) @ ktvbd                         # [H]
    z = (x64 @ u + c) / 8.0                                   # [L, H]
    p = 1.0 / (1.0 + np.exp(-z))
    return dict(v=v, xvt=xv.T, ktvbdt=ktvbd.T, ut=u.T, c=c / 8.0, out=p.T)


B = 8
_BDM = np.zeros((H, D), dtype=np.float32)
for _h in range(H):
    _BDM[_h, 64 * _h:64 * _h + 64] = 1.0
_cache = {}

def _get_nc(xbf16=True):
    if "nc" not in _cache:
        _cache["nc"] = build()
    return _cache["nc"]


def _make_cst(Wq, Wk, Wv, bq, bk, bv):
    cstm = np.zeros((128, 176), dtype=np.float32)
    for k in range(NDT):
        cstm[:, 16 * k:16 * k + 16] = Wv[128 * k:128 * k + 128, :]
    bqt = bq.reshape(8, 128).T
    cstm[:, 128:144:2] = bqt
    cstm[:, 129:144:2] = bqt
    cstm[:, 144:160] = np.broadcast_to(bv[None, :], (128, 16))
    cstm[0:16, 160] = bv
    return np.ascontiguousarray(cstm)


def kernel(x, mask, Wq, bq, Wk, bk, Wv, bv, xbf16=True):
    from concourse.bass_utils import run_bass_kernel_spmd
    x = np.asarray(x, dtype=np.float32)
    mask_f = np.asarray(mask).astype(np.float32)
    Wq = np.ascontiguousarray(np.asarray(Wq, dtype=np.float32))
    Wk = np.ascontiguousarray(np.asarray(Wk, dtype=np.float32))
    Wv = np.ascontiguousarray(np.asarray(Wv, dtype=np.float32))
    bq = np.asarray(bq, dtype=np.float32)
    bk = np.asarray(bk, dtype=np.float32)
    bv = np.asarray(bv, dtype=np.float32)
    nc = _get_nc()
    cstm = _make_cst(Wq, Wk, Wv, bq, bk, bv)
    bkb_ = np.ascontiguousarray(np.broadcast_to(bk[None, :], (H, D)))
    in_maps = []
    for b in range(B):
        in_maps.append({
            "x": np.ascontiguousarray(x[b]),
            "wq": Wq, "wk": Wk,
            "cst": cstm, "bkb": bkb_, "bdm": _BDM,
        })
    res = run_bass_kernel_spmd(nc, in_maps, core_ids=list(range(B)))
    out = np.stack([res.results[b]["out"] for b in range(B)], axis=0)
    out = out * mask_f[:, None, :]
    return out.astype(np.float32)


# revision 19
# speedup vs baseline: 1.1720x; 1.0066x over previous
"""MultiHeadSelectiveAttention TRN2 kernel: FULL inputs -> FULL output.

Shards batch (B=8) across 8 NeuronCores (data-parallel, one batch element
per core). Per batch b, using the value-head-dim-1 collapse:
    v   = x Wv + bv                        [L, H]
    xv  = x^T v                            [D, H]
    ktv = blockdiag_mask(Wk^T xv + bk (x) sum_l v)   [D, H]
    U   = Wq ktv ;  c[h] = bq . ktv[:, h]
    out = sigmoid((x U + c)/8)^T * mask    [H, L]
identical in exact arithmetic to the reference attention.

v5: single-rounded math, fp16 x-path. x streams alone on the sync HWDGE
queue (done ~48us) and is cast f32->fp16 on the vector engine; transposes
run in fp16 (FWL-fast weight loads, 1 cyc/row). Weights stream on the
scalar HWDGE queue (wk prefetched, wq JIT-transposed in phase B). s3/s4
stay f32r; v/xv/z passes are fp16 (half-ulp 2^-11, ~4x tighter than bf16).
Mask applied on host (a no-op for all-ones masks).
"""
import sys
sys.path.insert(0, '/opt/trn_rl_repo')
from contextlib import ExitStack
import numpy as np
import concourse.bass as bass
import concourse.tile as tile
import concourse.mybir as mybir
from concourse.tile import ScopedClock
from concourse.masks import make_identity

f32 = mybir.dt.float32
f32r = mybir.dt.float32r
bf16 = mybir.dt.bfloat16
fp16 = mybir.dt.float16
Copy = mybir.ActivationFunctionType.Copy
Sigmoid = mybir.ActivationFunctionType.Sigmoid

L, D, H = 4096, 1024, 16
NLT, NDT = L // 128, D // 128   # 32, 8
BLK = 4                          # l-tiles per block
NBLK = NLT // BLK                # 8

_wait_fix_counter = [0]
SPLIT_WAITS = [True]

def _split_multi_waits(nc):
    for f in nc.m.functions:
        for bb in f.blocks:
            new_insts = []
            for inst in bb.instructions:
                si = getattr(inst, 'sync_info', None)
                if si is not None and len(si.on_wait) > 1:
                    waits = list(si.on_wait)
                    for w in waits[:-1]:
                        _wait_fix_counter[0] += 1
                        nop = mybir.InstNoOp(
                            name=f"waitfix-{_wait_fix_counter[0]}",
                            engine=inst.engine, opcode="NoOp", ins=[], outs=[],
                            sync_info=mybir.SyncInfo(on_wait=[w], on_update=[]),
                        )
                        new_insts.append(nop)
                    inst.sync_info = mybir.SyncInfo(
                        on_wait=[waits[-1]], on_update=list(si.on_update))
                new_insts.append(inst)
            bb.instructions[:] = new_insts

def _drain_and_barrier_split(self, tick_clock, wait_clock):
    nc = self.nc
    probe = nc.sync.nop()
    wait_clock.add_sem_waits(probe.ins, ScopedClock({None: tick_clock.global_clock}))
    nc.sync.drain()
    nc.all_engine_barrier()
    assert self.sems is not None
    popped = nc._tile_sem_poison_stack.pop()
    assert popped is self._sem_poison
    nc.clear_and_free_semaphores(list(self.sems.allocated().values()))
    nc.all_engine_barrier()
    if SPLIT_WAITS[0]:
        _split_multi_waits(nc)

tile.TileContext._drain_and_barrier = _drain_and_barrier_split


def build():
    nc = bass.Bass(trn_type="TRN2")
    x = nc.dram_tensor("x", [L, D], f32r, kind="ExternalInput")
    wk = nc.dram_tensor("wk", [D, D], f32r, kind="ExternalInput")
    wq = nc.dram_tensor("wq", [D, D], f32r, kind="ExternalInput")
    # cst packs: cols 0:128 Wv d-tiles; 128:144 bq (dup pairs);
    # 144:160 bv bcast; col 160 rows 0:16 bv column
    cst = nc.dram_tensor("cst", [128, 176], f32, kind="ExternalInput")
    bkb = nc.dram_tensor("bkb", [H, D], f32, kind="ExternalInput")
    bdm = nc.dram_tensor("bdm", [H, D], f32, kind="ExternalInput")
    out = nc.dram_tensor("out", [H, L], f32, kind="ExternalOutput")

    with ExitStack() as ctx:
        tc = ctx.enter_context(tile.TileContext(nc))
        konst = ctx.enter_context(tc.tile_pool(name="konst", bufs=1))
        xtrp = ctx.enter_context(tc.tile_pool(name="xtr", bufs=1))
        pers = ctx.enter_context(tc.tile_pool(name="pers", bufs=1))
        wkp = ctx.enter_context(tc.tile_pool(name="wkp", bufs=1))
        ps_xv = ctx.enter_context(tc.tile_pool(name="ps_xv", bufs=1, space="PSUM"))

        # ---------------- constants ----------------
        cstt = konst.tile([128, 176], f32)
        nc.scalar.dma_start(cstt[:], cst[:, :])
        ident = konst.tile([128, 128], f32)
        make_identity(nc, ident[:])
        identh = konst.tile([128, 128], fp16)
        nc.vector.tensor_copy(identh[:], ident[:])
        # wk/wqh tiles filled during phase A (DMA slack)
        wkt = [wkp.tile([128, D], f32r, name=f"wkp{k}", tag=f"wkt{k}")
               for k in range(NDT)]
        wqtp = ctx.enter_context(tc.tile_pool(name="wqtp", bufs=1))
        wqt = [wqtp.tile([128, D], fp16, name=f"wqt{c}", tag=f"wqt{c}")
               for c in range(NDT)]
        wvh = konst.tile([128, 128], fp16)
        nc.vector.tensor_copy(wvh[:], cstt[:, 0:128])
        wv_d = [wvh[:, 16 * k:16 * k + 16] for k in range(NDT)]
        bqc = konst.tile([128, 16], fp16)
        nc.vector.tensor_copy(bqc[:], cstt[:, 128:144])
        bvtb = konst.tile([128, H], fp16)
        nc.vector.tensor_copy(bvtb[:], cstt[:, 144:160])

        # persistent: fp16 x^T for the v and z passes
        xtr = [xtrp.tile([128, L], fp16, name=f"xtr{d}", tag=f"xtr{d}")
               for d in range(NDT)]
        xv_ps = [ps_xv.tile([16, 512], f32, name=f"xv{c}", tag=f"xv{c}")
                 for c in range(2)]
        prep = ctx.enter_context(tc.tile_pool(name="prep", bufs=1))
        bkt = pers.tile([H, D], f32)
        bdmT = prep.tile([H, D], f32, name="bdmT", tag="bdmT")
        nxv = [0, 0]
        svps = []

        # ---------------- PHASE A ----------------
        with tc.tile_pool(name="xnatp", bufs=3) as xnatp, \
             tc.tile_pool(name="xhp", bufs=2) as xhp, \
             tc.tile_pool(name="wqnp", bufs=2) as wqnp, \
             tc.tile_pool(name="sbA", bufs=2) as sbA, \
             tc.tile_pool(name="vnp", bufs=3) as vnp, \
             tc.tile_pool(name="ps_tr", bufs=2, space="PSUM") as ps_tr, \
             tc.tile_pool(name="ps_q", bufs=2, space="PSUM") as ps_q, \
             tc.tile_pool(name="ps_v", bufs=1, space="PSUM") as ps_v, \
             tc.tile_pool(name="ps_f", bufs=1, space="PSUM") as ps_f:
            prev_p2 = [None]

            def p2_step(m):
                # one of 8 P2 matmuls for the previous block
                pblk, vsum_p, xh_p = prev_p2[0]
                j, c = m // 2, m % 2
                nxv[c] += 1
                nc.tensor.matmul(
                    xv_ps[c][:],
                    vsum_p[:, 16 * j:16 * j + 16],
                    xh_p[:, D * j + 512 * c:D * j + 512 * c + 512],
                    start=(nxv[c] == 1), stop=(nxv[c] == NLT))

            for blk in range(NBLK):
                xblk = xnatp.tile([128, BLK * D], f32r, tag="xnat")
                nc.sync.dma_start(
                    xblk[:].rearrange("p (j d) -> p j d", j=BLK),
                    x[512 * blk:512 * blk + 512, :]
                    .rearrange("(j p) d -> p j d", p=128))
                # weight prefetch for the PREVIOUS index runs late in this
                # block (keeps the head of the DMA pipe clear for x)
                wc = wqnp.tile([128, D], f32r, tag="wqc")
                wch = wqnp.tile([128, D], fp16, tag="wqch")
                xh = xhp.tile([128, BLK * D], fp16, tag="xh")
                for j in range(BLK):
                    nc.vector.tensor_copy(
                        xh[:, D * j:D * (j + 1)],
                        xblk[:, D * j:D * (j + 1)].bitcast(f32))
                nc.scalar.dma_start(wkt[blk][:], wk[128 * blk:128 * blk + 128, :])
                nc.scalar.dma_start(wc[:], wq[128 * blk:128 * blk + 128, :])
                nc.vector.tensor_copy(wch[:], wc[:].bitcast(f32))
                xnat = [xh[:, D * j:D * (j + 1)] for j in range(BLK)]
                # x transposes (fp16) with P1 of this block lagged 2 d-groups
                # and P2 of the previous block interleaved (keeps PE warm)
                psv = ps_v.tile([16, 512], f32, tag="v")
                def p1_step(d):
                    nc.tensor.matmul(
                        psv[:], wv_d[d], xtr[d][:, 512 * blk:512 * blk + 512],
                        start=(d == 0), stop=(d == NDT - 1))
                for d in range(NDT):
                    ps = ps_tr.tile([128, 512], fp16, tag="tr")
                    for j in range(BLK):
                        nc.tensor.matmul(
                            ps[:, 128 * j:128 * j + 128],
                            xnat[j][:, 128 * d:128 * d + 128],
                            identh[:],
                            start=True, stop=True, is_transpose=True,
                            skip_group_check=True)
                    if d % 2 == 0:
                        nc.scalar.copy(xtr[d][:, 512 * blk:512 * blk + 512], ps[:])
                    else:
                        nc.vector.tensor_copy(
                            xtr[d][:, 512 * blk:512 * blk + 512], ps[:])
                    psw = ps_q.tile([128, 128], fp16, tag="qw")
                    nc.tensor.matmul(
                        psw[:], wch[:, 128 * d:128 * d + 128], identh[:],
                        start=True, stop=True, is_transpose=True,
                        skip_group_check=True)
                    if d % 2 == 0:
                        nc.vector.tensor_copy(
                            wqt[d][:, 128 * blk:128 * blk + 128], psw[:])
                    else:
                        nc.scalar.copy(
                            wqt[d][:, 128 * blk:128 * blk + 128], psw[:])
                    if prev_p2[0] is not None:
                        p2_step(d)
                    if d >= 3:
                        p1_step(d - 3)
                p1_step(NDT - 3)
                p1_step(NDT - 2)
                p1_step(NDT - 1)
                vts = sbA.tile([16, 512], fp16, tag="vts")
                svp = sbA.tile([16, 1], f32, name="svp", tag=f"svp{blk}", bufs=1)
                nc.scalar.activation(vts[:], psv[:], Copy, accum_out=svp[:])
                svps.append(svp)
                # fold-transpose to v natural [128, 16] per l-tile
                psf = ps_f.tile([128, BLK * 16], fp16, tag="vf")
                for j in range(BLK):
                    nc.tensor.matmul(
                        psf[:, 16 * j:16 * j + 16],
                        vts[:, 128 * j:128 * j + 128],
                        identh[0:16, 0:16],
                        start=True, stop=True, is_transpose=True,
                        skip_group_check=True)
                vsum = vnp.tile([128, BLK * 16], fp16, tag="vsum")
                vs3 = vsum[:].rearrange("p (j h) -> p j h", j=BLK)
                pf3 = psf[:].rearrange("p (j h) -> p j h", j=BLK)
                nc.vector.tensor_add(
                    vs3, pf3, bvtb[:].unsqueeze(1).broadcast_to([128, BLK, H]))
                prev_p2[0] = (blk, vsum, xh)
            for m in range(8):
                p2_step(m)

        # xv drain + sv
        xvt = pers.tile([H, D], f32, tag="xvt")
        for c in range(2):
            nc.scalar.copy(xvt[:, 512 * c:512 * c + 512], xv_ps[c][:])
        sv = pers.tile([H, 1], f32, tag="sv")
        nc.vector.tensor_add(sv[:], svps[0][:], svps[1][:])
        for b in range(2, NBLK):
            nc.vector.tensor_add(sv[:], sv[:], svps[b][:])
        bvL = pers.tile([H, 1], f32, tag="bvL")
        nc.scalar.mul(bvL[:], cstt[0:16, 160:161], float(L))
        nc.vector.tensor_add(sv[:], sv[:], bvL[:])

        # ---------------- PHASE B ----------------
        with tc.tile_pool(name="sbB", bufs=2) as sbB:
            nc.scalar.dma_start(bkt[:], bkb[:, :])
            nc.scalar.dma_start(bdmT[:], bdm[:, :])
            # xv natural tiles [128, 16] f32r per d-tile
            xvp = []
            with tc.tile_pool(name="ps_m1", bufs=2, space="PSUM") as ps_m1:
                for k in range(NDT):
                    psm = ps_m1.tile([128, 16], f32, tag="m1")
                    nc.tensor.matmul(
                        psm[:], xvt[:, 128 * k:128 * k + 128], ident[0:16, 0:16],
                        start=True, stop=True, is_transpose=True,
                        skip_group_check=True)
                    p = prep.tile([128, 16], f32r, name=f"xvp{k}", tag=f"xvp{k}")
                    nc.scalar.copy(p[:], psm[:])
                    xvp.append(p)
            # s3: ktv_full^T = xv^T Wk (wk prefetched)
            with tc.tile_pool(name="ps_s3", bufs=1, space="PSUM") as ps_s3:
                ps3 = [ps_s3.tile([16, 512], f32, name=f"s3{c}", tag=f"s3{c}")
                       for c in range(2)]
                for k in range(NDT):
                    for c in range(2):
                        nc.tensor.matmul(
                            ps3[c][:], xvp[k][:],
                            wkt[k][:, 512 * c:512 * c + 512],
                            start=(k == 0), stop=(k == NDT - 1))
                ktvt = sbB.tile([H, D], f32, tag="ktvt", bufs=1)
                nc.scalar.activation(ktvt[:], bkt[:], Copy, scale=sv[:])
                for c in range(2):
                    sl = ktvt[:, 512 * c:512 * c + 512]
                    nc.vector.tensor_add(sl, sl, ps3[c][:])
                nc.vector.tensor_mul(ktvt[:], ktvt[:], bdmT[:])
            # ktv_bd natural tiles + c accumulation
            ktvp = []
            with tc.tile_pool(name="ps_m2", bufs=2, space="PSUM") as ps_m2, \
                 tc.tile_pool(name="ps_c", bufs=1, space="PSUM") as ps_c:
                for k in range(NDT):
                    psm = ps_m2.tile([128, 16], f32, tag="m2")
                    nc.tensor.matmul(
                        psm[:], ktvt[:, 128 * k:128 * k + 128], ident[0:16, 0:16],
                        start=True, stop=True, is_transpose=True,
                        skip_group_check=True)
                    p = prep.tile([128, 16], fp16, name=f"ktvp{k}", tag=f"ktvp{k}")
                    nc.scalar.copy(p[:], psm[:])
                    ktvp.append(p)
                psc = ps_c.tile([16, 2], f32, tag="c", bufs=1)
                for k in range(NDT):
                    nc.tensor.matmul(
                        psc[:], ktvp[k][:], bqc[:, 2 * k:2 * k + 2],
                        start=(k == 0), stop=(k == NDT - 1))
                cdiv8 = pers.tile([H, 1], f32, tag="cdiv8")
                nc.scalar.mul(cdiv8[:], psc[:, 0:1], 0.125)
            # s4: U^T accumulation (wqt resident from phase A)
            with tc.tile_pool(name="ps_s4", bufs=1, space="PSUM") as ps_s4:
                ps4 = [ps_s4.tile([16, 512], f32, name=f"s4{c}", tag=f"s4{c}")
                       for c in range(2)]
                for b in range(NDT):
                    for c in range(2):
                        nc.tensor.matmul(
                            ps4[c][:], ktvp[b][:],
                            wqt[b][:, 512 * c:512 * c + 512],
                            start=(b == 0), stop=(b == NDT - 1))
                ut = sbB.tile([H, D], f32, name="ut", tag="ut", bufs=1)
                for c in range(2):
                    nc.scalar.copy(ut[:, 512 * c:512 * c + 512], ps4[c][:])
            # U natural tiles [128, 16] per d-tile (bf16 for the z pass)
            upr = []
            with tc.tile_pool(name="ps_m3", bufs=2, space="PSUM") as ps_m3:
                for d in range(NDT):
                    psm = ps_m3.tile([128, 16], f32, tag="m3")
                    nc.tensor.matmul(
                        psm[:], ut[:, 128 * d:128 * d + 128], ident[0:16, 0:16],
                        start=True, stop=True, is_transpose=True,
                        skip_group_check=True)
                    p = prep.tile([128, 16], fp16, name=f"upr{d}", tag=f"upr{d}")
                    nc.vector.tensor_copy(p[:], psm[:])
                    upr.append(p)
            # P5: z^T chunks + sigmoid into one tile + single store
            sgall = sbB.tile([H, L], f32, name="sgall", tag="sgall", bufs=1)
            with tc.tile_pool(name="ps_5", bufs=2, space="PSUM") as ps_5:
                for ch in range(8):
                    ps5 = ps_5.tile([16, 512], f32, tag="s5")
                    for d in range(NDT):
                        nc.tensor.matmul(
                            ps5[:], upr[d][:], xtr[d][:, 512 * ch:512 * ch + 512],
                            start=(d == 0), stop=(d == NDT - 1))
                    nc.scalar.activation(sgall[:, 512 * ch:512 * ch + 512],
                                         ps5[:], Sigmoid,
                                         bias=cdiv8[:], scale=0.125)
                    if ch == 6:
                        nc.sync.dma_start(out[:, 0:512 * 7], sgall[:, 0:512 * 7])
            nc.sync.dma_start(out[:, 512 * 7:L], sgall[:, 512 * 7:L])
    return nc


def ref_numpy(x, wq, wk, wv, bq, bk, bv):
    """f64 reference of the decomposed math for per-stage validation."""
    x64 = x.astype(np.float64)
    v = x64 @ wv.astype(np.float64) + bv.astype(np.float64)   # [L, H]
    xv = x64.T @ v                                            # [D, H]
    ktvfull = wk.astype(np.float64).T @ xv                    # [D, H]
    sv = v.sum(axis=0)                                        # [H]
    ktvfull = ktvfull + np.outer(bk.astype(np.float64), sv)
    bd = np.zeros((D, H))
    for h in range(H):
        bd[64 * h:64 * h + 64, h] = 1.0
    ktvbd = ktvfull * bd
    u = wq.astype(np.float64) @ ktvbd                         # [D, H]
    c = bq.astype(np.float64) @ ktvbd                         # [H]
    z = (x64 @ u + c) / 8.0                                   # [L, H]
    p = 1.0 / (1.0 + np.exp(-z))
    return dict(v=v, xvt=xv.T, ktvbdt=ktvbd.T, ut=u.T, c=c / 8.0, out=p.T)


B = 8
_BDM = np.zeros((H, D), dtype=np.float32)
for _h in range(H):
    _BDM[_h, 64 * _h:64 * _h + 64] = 1.0
_cache = {}

def _get_nc(xbf16=True):
    if "nc" not in _cache:
        _cache["nc"] = build()
    return _cache["nc"]


def _make_cst(Wq, Wk, Wv, bq, bk, bv):
    cstm = np.zeros((128, 176), dtype=np.float32)
    for k in range(NDT):
        cstm[:, 16 * k:16 * k + 16] = Wv[128 * k:128 * k + 128, :]
    bqt = bq.reshape(8, 128).T
    cstm[:, 128:144:2] = bqt
    cstm[:, 129:144:2] = bqt
    cstm[:, 144:160] = np.broadcast_to(bv[None, :], (128, 16))
    cstm[0:16, 160] = bv
    return np.ascontiguousarray(cstm)


def kernel(x, mask, Wq, bq, Wk, bk, Wv, bv, xbf16=True):
    from concourse.bass_utils import run_bass_kernel_spmd
    x = np.asarray(x, dtype=np.float32)
    mask_f = np.asarray(mask).astype(np.float32)
    Wq = np.ascontiguousarray(np.asarray(Wq, dtype=np.float32))
    Wk = np.ascontiguousarray(np.asarray(Wk, dtype=np.float32))
    Wv = np.ascontiguousarray(np.asarray(Wv, dtype=np.float32))
    bq = np.asarray(bq, dtype=np.float32)
    bk = np.asarray(bk, dtype=np.float32)
    bv = np.asarray(bv, dtype=np.float32)
    nc = _get_nc()
    cstm = _make_cst(Wq, Wk, Wv, bq, bk, bv)
    bkb_ = np.ascontiguousarray(np.broadcast_to(bk[None, :], (H, D)))
    in_maps = []
    for b in range(B):
        in_maps.append({
            "x": np.ascontiguousarray(x[b]),
            "wq": Wq, "wk": Wk,
            "cst": cstm, "bkb": bkb_, "bdm": _BDM,
        })
    res = run_bass_kernel_spmd(nc, in_maps, core_ids=list(range(B)))
    out = np.stack([res.results[b]["out"] for b in range(B)], axis=0)
    out = out * mask_f[:, None, :]
    return out.astype(np.float32)
